# revision 8
# baseline (speedup 1.0000x reference)
"""Trainium2 kernel for nn_Group_10: 3x3 replicate-pad conv [4,512,32,32] ->
[4,9728,32,32] (+bias) followed by a per-64-channel-chunk pixel shuffle to
[4,152,256,256].

Sharding: output channels across 8 cores (19 chunks of 64 = 1216 couts each,
padded to 1280 = 10 PE tiles of 128).

Key trick: the pixel shuffle is a pure per-chunk permutation
    out[p, q] = y[cc, h, w],  p = (cc>>1)*8 + (w&7),  q = (cc&1)*128 + 4h + (w>>3)
so the matmul's moving operand visits pixels in order f' = (w&7)*128 + 4h + (w>>3)
(a 3-dim strided AP over the padded image), PSUM comes out already in shuffled
free order, and (with PE output partitions permuted as m = chunkbit*64 +
parity*32 + (cc>>1)) the store to DRAM is a single strided DMA with 512B
contiguous runs.  Matmuls run in float32r (full-rate fp32 PE mode); the BIR
verifier requires every producer feeding an fp32r matmul to write float32r,
so the x/w staging tiles are declared f32r and the DRAM-side APs bitcast.
"""

import numpy as np
from contextlib import ExitStack

import concourse.bass as bass
import concourse.mybir as mybir
import concourse.tile as tile
from concourse import bacc
from concourse.bass_utils import run_bass_kernel_spmd

F32 = mybir.dt.float32
F32R = mybir.dt.float32r

N_CORES = 8
B = 4
CIN = 512
H = W_ = 32
COUT = 9728
NCHUNK = COUT // 64            # 152
CH_PER_CORE = NCHUNK // N_CORES  # 19
COUT_CORE = COUT // N_CORES    # 1216
NTILES = 10                    # 1216 padded to 1280 = 10 tiles of 128
HP = WP = 34                   # replicate-padded image
PIX = HP * WP                  # 1156
NCT = CIN // 128               # 4 cin tiles

# within-tile PE output-partition permutation:
#   partition m = chunkbit*64 + parity*32 + cchalf  <->  cout_in_tile =
#   chunkbit*64 + 2*cchalf + parity
_m = np.arange(128)
_chunkbit, _rem = np.divmod(_m, 64)
_parity, _cchalf = np.divmod(_rem, 32)
COUT_IN_TILE = (_chunkbit * 64 + 2 * _cchalf + _parity).astype(np.int64)  # [128]

_nc_cache = None


def _build_nc(rep=1):
    """rep>1 wraps the body in an on-device For_i loop — identical I/O
    signature, used by test.py to measure per-iteration HW time by
    differencing wall-clocks against the rep=1 build."""
    nc = bacc.Bacc("TRN2", target_bir_lowering=False, debug=False,
                   num_devices=N_CORES)

    xp = nc.dram_tensor("xp", [B, CIN, HP, WP], F32, kind="ExternalInput")
    w = nc.dram_tensor("w", [NTILES, 128, NCT, 9, 128], F32,
                       kind="ExternalInput")
    bias = nc.dram_tensor("bias", [128, NTILES], F32, kind="ExternalInput")
    out = nc.dram_tensor("out", [B, CH_PER_CORE, 256, 256], F32,
                         kind="ExternalOutput")

    with ExitStack() as ctx:
        tc = ctx.enter_context(tile.TileContext(nc))
        xpool = ctx.enter_context(tc.tile_pool(name="xpool", bufs=1))
        wpool = ctx.enter_context(tc.tile_pool(name="wpool", bufs=2))
        opool = ctx.enter_context(tc.tile_pool(name="opool", bufs=3))
        bpool = ctx.enter_context(tc.tile_pool(name="bpool", bufs=1))
        ppool = ctx.enter_context(tc.tile_pool(name="ppool", bufs=3,
                                               space="PSUM"))

        def body():
            # x resident in SBUF: partition = cin%128, free = (n, ct) slabs
            # of 1156 pixels in natural (h, w) padded order.
            x_sb = xpool.tile([128, B * NCT * PIX], F32R)
            xrow = x_sb.ap[0][0]
            xt = x_sb.tensor
            xoff0 = x_sb.offset
            for n in range(B):
                for ct in range(NCT):
                    dst = bass.AP(xt, xoff0 + (n * NCT + ct) * PIX,
                                  [[xrow, 128], [1, PIX]])
                    src = bass.AP(xp, (n * CIN + ct * 128) * PIX,
                                  [[PIX, 128], [1, PIX]]).bitcast(F32R)
                    nc.sync.dma_start(dst, src)

            bias_sb = bpool.tile([128, NTILES], F32)
            nc.sync.dma_start(bias_sb, bias[:])

            w_ap = w[:]
            for t in range(NTILES):
                w_sb = wpool.tile([128, NCT * 9 * 128], F32R)
                nc.sync.dma_start(w_sb, w_ap[t].bitcast(F32R))
                wrow = w_sb.ap[0][0]
                wt = w_sb.tensor
                woff = w_sb.offset
                for n in range(B):
                    psum = ppool.tile([128, 1024], F32)
                    for bk in range(2):          # PSUM bank = r-halves (w&7)
                        for tap in range(9):
                            dy, dx = divmod(tap, 3)
                            for ct in range(NCT):
                                rhs = bass.AP(
                                    xt,
                                    xoff0 + (n * NCT + ct) * PIX + dy * WP
                                    + dx + 4 * bk,
                                    [[xrow, 128], [1, 4], [WP, 32], [8, 4]],
                                )
                                lhsT = bass.AP(
                                    wt, woff + (ct * 9 + tap) * 128,
                                    [[wrow, 128], [1, 128]],
                                )
                                nc.tensor.matmul(
                                    psum[:, 512 * bk:512 * (bk + 1)], lhsT,
                                    rhs,
                                    start=(tap == 0 and ct == 0),
                                    stop=(tap == 8 and ct == 3),
                                )
                    o_sb = opool.tile([128, 1024], F32)
                    nc.vector.tensor_scalar_add(o_sb, psum,
                                                bias_sb[:, t:t + 1])
                    orow = o_sb.ap[0][0]
                    nchunks = 2 if t < NTILES - 1 else 1
                    for cb in range(nchunks):
                        src = bass.AP(o_sb.tensor,
                                      o_sb.offset + cb * 64 * orow,
                                      [[orow, 64], [128, 8], [1, 128]])
                        base = (n * CH_PER_CORE + 2 * t + cb) * 65536
                        dst = bass.AP(out, base,
                                      [[128, 2], [2048, 32], [256, 8],
                                       [1, 128]])
                        nc.sync.dma_start(dst, src)

        if rep == 1:
            body()
        else:
            with tc.For_i(0, rep):
                body()

    nc.compile()
    return nc


def _host_prep(x, W, b):
    """Build per-core input maps."""
    xpad = np.pad(np.asarray(x, dtype=np.float32),
                  ((0, 0), (0, 0), (1, 1), (1, 1)), mode="edge")
    xpad = np.ascontiguousarray(xpad)
    W = np.asarray(W, dtype=np.float32)
    b = np.asarray(b, dtype=np.float32)

    in_maps = []
    for i in range(N_CORES):
        Ws = W[i * COUT_CORE:(i + 1) * COUT_CORE]          # [1216,512,3,3]
        Wp = np.zeros((NTILES * 128, CIN, 3, 3), np.float32)
        Wp[:COUT_CORE] = Ws
        gather = (np.arange(NTILES)[:, None] * 128 +
                  COUT_IN_TILE[None, :])                   # [10,128]
        Wg = Wp[gather]                                    # [10,128(m),512,3,3]
        Wg = Wg.reshape(NTILES, 128, NCT, 128, 9)          # [t,m,ct,p,tap]
        w_dev = np.ascontiguousarray(Wg.transpose(0, 3, 2, 4, 1))  # [t,p,ct,tap,m]

        bp = np.zeros((NTILES * 128,), np.float32)
        bp[:COUT_CORE] = b[i * COUT_CORE:(i + 1) * COUT_CORE]
        bias_dev = np.ascontiguousarray(bp[gather].T)      # [128,10]

        in_maps.append({"xp": xpad, "w": w_dev, "bias": bias_dev})
    return in_maps


def _run(in_maps, trace=False):
    global _nc_cache
    if _nc_cache is None:
        _nc_cache = _build_nc()
    return run_bass_kernel_spmd(_nc_cache, in_maps,
                                core_ids=list(range(N_CORES)), trace=trace)


def kernel(x, W, b):
    in_maps = _host_prep(x, W, b)
    res = _run(in_maps)
    outs = [res.results[i]["out"] for i in range(N_CORES)]  # [4,19,256,256]
    full = np.concatenate(outs, axis=1)                     # [4,152,256,256]
    return full
